# revision 8
# baseline (speedup 1.0000x reference)
"""NeRF-style positional encoding kernel for Trainium2 (8 NeuronCores).

out[n, 2j]   = cos(x[n] * freqs[j])
out[n, 2j+1] = sin(x[n] * freqs[j])     freqs[j] = fl(pi * exp2(j)) as the
                                        neuron device computes them.

Bit-exact replication of the neuronx-cc lowering of jnp.sin/jnp.cos:
    sin(v): t = RN(RN(v + PI) * INV2PI); k = floor(t)  [rne-convert + is_gt fixup]
            r = RN(v + RN(k * -TWOPI)); out = ActSin(r)
    cos(v): same chain applied to y = RN(v + HALFPI)

Sharding: pure data-parallel across 8 cores along n.
"""
import numpy as np

import concourse.bacc as bacc
import concourse.tile as tile
import concourse.mybir as mybir
from concourse.bass_utils import run_bass_kernel_spmd

N_TOTAL = 4194304
N_CORES = 8
N_PER_CORE = N_TOTAL // N_CORES     # 524288
D = 32

P = 128                             # partitions
F = 256                             # x elements per partition per tile
TILE_ELEMS = P * F                  # 32768
N_TILES = N_PER_CORE // TILE_ELEMS  # 16
G = 8                               # j's per group
N_GROUPS = D // G                   # 4
GF = G * F                          # 2048

# Device-computed freqs bits (pi * exp2(arange(32)) evaluated by neuronx-cc
# on trn2 -- the exp2 ACT table is not exact, so these differ from
# fl(pi)*2^j for most j).
FREQS_BITS = [
    1078530011, 1086918608, 1095307227, 1103695832, 1112084378, 1120473016,
    1128861658, 1137250267, 1145638851, 1154027401, 1162416086, 1170804699,
    1179193302, 1187581820, 1195970503, 1204359125, 1212747739, 1221136313,
    1229524901, 1237913555, 1246302171, 1254690773, 1263079269, 1271467979,
    1279856603, 1288245214, 1296633765, 1305022388, 1313411035, 1321799649,
    1330188214, 1338576773,
]
FREQS = np.array(FREQS_BITS, dtype=np.uint32).view(np.float32)

PI = float(np.float32(3.1415927410125732))       # 0x40490FDB
INV2PI = float(np.float32(0.15915493667125702))  # 0x3E22F983
NTWOPI = float(np.float32(-6.2831854820251465))  # 0xC0C90FDB
HALFPI = float(np.float32(1.5707963705062866))   # 0x3FC90FDB

_NC_CACHE = {}


def build_nc(repeats: int = 1, n_tiles: int = N_TILES, n_devices: int = N_CORES):
    f32 = mybir.dt.float32
    i32 = mybir.dt.int32
    u8 = mybir.dt.uint8
    A = mybir.AluOpType
    Sin = mybir.ActivationFunctionType.Sin
    Ident = mybir.ActivationFunctionType.Identity

    n_elems = n_tiles * TILE_ELEMS
    nc = bacc.Bacc("TRN2", target_bir_lowering=False, debug=False,
                   num_devices=n_devices)
    x = nc.dram_tensor("x", [n_elems], f32, kind="ExternalInput")
    out = nc.dram_tensor("out", [n_elems, 2 * D], f32, kind="ExternalOutput")

    # DRAM views: tile ti covers rows [ti*32768, (ti+1)*32768); partition p
    # holds rows ti*32768 + p*256 .. +256 (each row = 64 contiguous f32).
    x_v = x.ap().rearrange("(t p c) -> t p c", p=P, c=F)
    out_v = out.ap().rearrange("(t p c) q -> t p (c q)", p=P, c=F)

    with tile.TileContext(nc) as tc:
        with (
            tc.tile_pool(name="io", bufs=2) as io_pool,
            tc.tile_pool(name="src", bufs=2) as src_pool,
            tc.tile_pool(name="work", bufs=4) as work_pool,
            tc.tile_pool(name="msk", bufs=2) as msk_pool,
            tc.tile_pool(name="cst", bufs=1) as cst_pool,
        ):
            hp = cst_pool.tile([P, 1], f32, tag="hp")
            nc.gpsimd.memset(hp[:], HALFPI)

            def path(src_t, out_t, parity):
                """floor-fixup reduction + ActSin on a [P, GF] source; writes
                the 8 per-j result columns into out_t at the given parity."""
                t = work_pool.tile([P, GF], f32, tag="work")
                nc.gpsimd.tensor_scalar(t[:], src_t[:], PI, INV2PI, A.add, A.mult)
                ki = work_pool.tile([P, GF], i32, tag="work")
                nc.gpsimd.tensor_scalar(ki[:], t[:], 0.0, 1.0, A.add, A.mult)
                mask = msk_pool.tile([P, GF], u8, tag="msk")
                nc.vector.tensor_tensor(mask[:], ki[:], t[:], A.is_gt)
                kfix = work_pool.tile([P, GF], f32, tag="work")
                nc.vector.scalar_tensor_tensor(kfix[:], mask[:], -1.0, ki[:],
                                               A.mult, A.add)
                r = work_pool.tile([P, GF], f32, tag="work")
                nc.vector.scalar_tensor_tensor(r[:], kfix[:], NTWOPI, src_t[:],
                                               A.mult, A.add)
                nc.scalar.activation(out_t, r[:].rearrange("p (e c) -> p e c", e=G),
                                     Sin)

            for ti in range(n_tiles * repeats):
                tix = ti % n_tiles
                xt = io_pool.tile([P, F], f32, tag="x")
                nc.sync.dma_start(xt[:], x_v[tix])

                ot = io_pool.tile([P, F * 2 * D], f32, tag="out")
                # [p, parity, j, c] view of the out tile
                ov = ot[:].rearrange("p (c e two) -> p two e c", e=D, two=2)

                for g in range(N_GROUPS):
                    ang = src_pool.tile([P, GF], f32, tag="src")
                    for i in range(G):
                        nc.vector.tensor_scalar_mul(
                            ang[:, i * F:(i + 1) * F], xt[:],
                            float(FREQS[g * G + i]))
                    # sin at odd columns
                    path(ang, ov[:, 1, g * G:(g + 1) * G, :], 1)
                    # y = ang + pi/2 on ACT; cos at even columns
                    y = src_pool.tile([P, GF], f32, tag="src")
                    nc.scalar.activation(y[:], ang[:], Ident, bias=hp[:], scale=1.0)
                    path(y, ov[:, 0, g * G:(g + 1) * G, :], 0)

                nc.sync.dma_start(out_v[tix], ot[:])

    nc.compile()
    return nc


def _get_nc(repeats: int = 1):
    if repeats not in _NC_CACHE:
        _NC_CACHE[repeats] = build_nc(repeats)
    return _NC_CACHE[repeats]


def kernel(x, d):
    assert int(d) == D
    x = np.ascontiguousarray(np.asarray(x, dtype=np.float32).reshape(N_TOTAL))
    xs = x.reshape(N_CORES, N_PER_CORE)
    nc = _get_nc()
    res = run_bass_kernel_spmd(
        nc, [{"x": xs[i]} for i in range(N_CORES)], core_ids=list(range(N_CORES)))
    out = np.empty((N_TOTAL, 2 * D), dtype=np.float32)
    for i in range(N_CORES):
        out[i * N_PER_CORE:(i + 1) * N_PER_CORE] = res.results[i]["out"]
    return out


def build_timing_nc(loop_iters: int, n_tiles: int = N_TILES):
    """Timing-only variant: output goes to internal DRAM (no D2H of 1 GiB),
    the whole n_tiles pass repeats loop_iters times in a hardware For_i loop,
    and a tiny dummy tensor is the only ExternalOutput."""
    f32 = mybir.dt.float32
    i32 = mybir.dt.int32
    u8 = mybir.dt.uint8
    A = mybir.AluOpType
    Sin = mybir.ActivationFunctionType.Sin
    Ident = mybir.ActivationFunctionType.Identity

    n_elems = n_tiles * TILE_ELEMS
    nc = bacc.Bacc("TRN2", target_bir_lowering=False, debug=False,
                   num_devices=N_CORES)
    x = nc.dram_tensor("x", [n_elems], f32, kind="ExternalInput")
    out = nc.dram_tensor("scratch_out", [n_elems, 2 * D], f32)
    tiny = nc.dram_tensor("tiny_out", [P, 1], f32, kind="ExternalOutput")

    x_v = x.ap().rearrange("(t p c) -> t p c", p=P, c=F)
    out_v = out.ap().rearrange("(t p c) q -> t p (c q)", p=P, c=F)

    with tile.TileContext(nc) as tc:
        with (
            tc.tile_pool(name="io", bufs=2) as io_pool,
            tc.tile_pool(name="src", bufs=2) as src_pool,
            tc.tile_pool(name="work", bufs=4) as work_pool,
            tc.tile_pool(name="msk", bufs=2) as msk_pool,
            tc.tile_pool(name="cst", bufs=1) as cst_pool,
        ):
            hp = cst_pool.tile([P, 1], f32, tag="hp")
            nc.gpsimd.memset(hp[:], HALFPI)

            def path(src_t, out_t):
                t = work_pool.tile([P, GF], f32, tag="work")
                nc.gpsimd.tensor_scalar(t[:], src_t[:], PI, INV2PI, A.add, A.mult)
                ki = work_pool.tile([P, GF], i32, tag="work")
                nc.gpsimd.tensor_scalar(ki[:], t[:], 0.0, 1.0, A.add, A.mult)
                mask = msk_pool.tile([P, GF], u8, tag="msk")
                nc.vector.tensor_tensor(mask[:], ki[:], t[:], A.is_gt)
                kfix = work_pool.tile([P, GF], f32, tag="work")
                nc.vector.scalar_tensor_tensor(kfix[:], mask[:], -1.0, ki[:],
                                               A.mult, A.add)
                r = work_pool.tile([P, GF], f32, tag="work")
                nc.vector.scalar_tensor_tensor(r[:], kfix[:], NTWOPI, src_t[:],
                                               A.mult, A.add)
                nc.scalar.activation(out_t, r[:].rearrange("p (e c) -> p e c", e=G),
                                     Sin)

            with tc.For_i(0, loop_iters, 1):
                for tix in range(n_tiles):
                    xt = io_pool.tile([P, F], f32, tag="x")
                    nc.sync.dma_start(xt[:], x_v[tix])
                    ot = io_pool.tile([P, F * 2 * D], f32, tag="out")
                    ov = ot[:].rearrange("p (c e two) -> p two e c", e=D, two=2)
                    for g in range(N_GROUPS):
                        ang = src_pool.tile([P, GF], f32, tag="src")
                        for i in range(G):
                            nc.vector.tensor_scalar_mul(
                                ang[:, i * F:(i + 1) * F], xt[:],
                                float(FREQS[g * G + i]))
                        path(ang, ov[:, 1, g * G:(g + 1) * G, :])
                        y = src_pool.tile([P, GF], f32, tag="src")
                        nc.scalar.activation(y[:], ang[:], Ident, bias=hp[:],
                                             scale=1.0)
                        path(y, ov[:, 0, g * G:(g + 1) * G, :])
                    nc.sync.dma_start(out_v[tix], ot[:])

            xd = io_pool.tile([P, 1], f32, tag="xd")
            nc.vector.memset(xd[:], 0.0)
            nc.sync.dma_start(tiny.ap(), xd[:])

    nc.compile()
    return nc
